# revision 16
# baseline (speedup 1.0000x reference)
"""Multi-head causal attention (B=4, S=2048, D=1024, H=16) on 8 trn2 NeuronCores.

Sharding: data-parallel over batch (4) x tensor-parallel over heads (2 groups
of 8).  Core c = (b, g) computes, for batch b, head group g.

v2: Q/K projections and QK^T (MM1) run in fp8e4m3 with DoubleRow perf mode
(2 contraction subtiles per matmul).  Q^T/K^T are stored "d-folded":
tile m holds heads 4m..4m+3; head slot j = h%4 occupies partitions
[32j, 32j+32) and the two 32-wide halves of head_dim sit at free slots
i in {0,1}.  The projection emits this layout directly via a host-side
column permutation of Wq/Wk (out partitions = W' columns).  Scaling:
x is shipped as fp8(16 x^T), W as fp8(8 W'), so psum = 128 (xW); the
bias add rescales to Q' = 8(xW+b).  MM1 psum is then 64 S and the exp
activation folds the /64 and /sqrt(HD) into one scale of 1/512.

exp ops cover both heads of a pair and up to 4 k-tiles ([128, <=1024]
from one double-buffered psum pair tile).  V proj / AV (MM2) / output
projection stay bf16 (fp8 there costs too much accuracy).  The causal
diagonal-block mask multiply runs on GPSIMD (Pool) to unload DVE.
PE slack inside the ACT-bound attention loop is filled by pumping
fine-grained projection / output-projection chunks between blocks.
Host sums the two per-batch partials and adds bo.
"""

import math

import numpy as np

B, S, D, H = 4, 2048, 1024, 16
HD = D // H          # 64
NCORES = 8
HPC = 8              # heads per core
DM = HPC * HD        # 512 mid-dims per core
NQT = S // 128       # 16 query tiles
KT_PER_EXP = 4       # k-tiles per exp block (both heads packed -> [128,1024])
VROW = HD + 1        # 65: per-head V columns incl. ones column

_CACHE = {}

# cost-model estimates used by the filler pump (ns)
_ACT_NS_PER_COL = 0.833
_ACT_OP_OVERHEAD_NS = 190.0
_PE_NS_PER_CY = 0.4167


def _build_program():
    import concourse.mybir as mybir
    import concourse.tile as tile
    from concourse import bacc

    f32 = mybir.dt.float32
    bf16 = mybir.dt.bfloat16
    fp8 = mybir.dt.float8e4
    DR = mybir.MatmulPerfMode.DoubleRow
    EXP = mybir.ActivationFunctionType.Exp
    MULT = mybir.AluOpType.mult
    ADD = mybir.AluOpType.add

    nc = bacc.Bacc("TRN2", target_bir_lowering=False, debug=False,
                   num_devices=NCORES)

    xq_d = nc.dram_tensor("xq", [D, S], fp8, kind="ExternalInput")
    xk_d = nc.dram_tensor("xk", [D, S], fp8, kind="ExternalInput")
    xv_d = nc.dram_tensor("xv", [D, S], bf16, kind="ExternalInput")
    wq_d = nc.dram_tensor("wq", [D, DM], fp8, kind="ExternalInput")
    wk_d = nc.dram_tensor("wk", [D, DM], fp8, kind="ExternalInput")
    wv_d = nc.dram_tensor("wv", [D, DM], bf16, kind="ExternalInput")
    bq_d = nc.dram_tensor("bq", [128, 4], f32, kind="ExternalInput")
    bk_d = nc.dram_tensor("bk", [128, 4], f32, kind="ExternalInput")
    bvb_d = nc.dram_tensor("bvb", [128, DM], f32, kind="ExternalInput")
    wo_d = nc.dram_tensor("wo", [DM, D], bf16, kind="ExternalInput")
    cmask_d = nc.dram_tensor("cmask", [128, 256], bf16, kind="ExternalInput")
    ident_d = nc.dram_tensor("ident", [128, 128], bf16, kind="ExternalInput")
    outT_d = nc.dram_tensor("outT", [D, S], f32, kind="ExternalOutput")

    with tile.TileContext(nc) as tc:
        with (
            tc.tile_pool(name="res", bufs=1) as res,     # long-lived tensors
            tc.tile_pool(name="wrk", bufs=1) as wrk,     # rotating work tiles
            tc.tile_pool(name="ps", bufs=1, space="PSUM") as ps,
        ):
            # ---- resident tensors -------------------------------------
            wq_sb = res.tile([128, 8 * DM], fp8, tag="wq_sb")
            wk_sb = res.tile([128, 8 * DM], fp8, tag="wk_sb")
            wv_sb = res.tile([128, 8 * DM], bf16, tag="wv_sb")
            wo_sb = res.tile([128, 4 * D], bf16, tag="wo_sb")
            bq_sb = res.tile([128, 4], f32, tag="bq_sb")
            bk_sb = res.tile([128, 4], f32, tag="bk_sb")
            bvb_sb = res.tile([128, DM], f32, tag="bvb_sb")
            cmask_sb = res.tile([128, 256], bf16, tag="cmask_sb")
            ident_sb = res.tile([128, 128], bf16, tag="ident_sb")
            # folded fp8 Q^T/K^T: [p=(slot,dm32), i, q]
            qT_sb = [res.tile([128, 2 * S], fp8, tag=f"qT{m}", name=f"qT{m}")
                     for m in range(2)]
            kT_sb = [res.tile([128, 2 * S], fp8, tag=f"kT{m}", name=f"kT{m}")
                     for m in range(2)]
            v_sb = res.tile([128, NQT * HPC * VROW], bf16, tag="v_sb")
            oT_sb = res.tile([128, 4 * S], bf16, tag="oT", name="oT")
            oT3 = oT_sb.rearrange("p (m s) -> p m s", m=4)

            wq5 = wq_sb.rearrange("p (k m i c) -> p k m i c", k=8, m=2, i=2)
            wk5 = wk_sb.rearrange("p (k m i c) -> p k m i c", k=8, m=2, i=2)
            qT3 = [t.rearrange("p (i s) -> p i s", i=2) for t in qT_sb]
            kT3 = [t.rearrange("p (i s) -> p i s", i=2) for t in kT_sb]
            v4 = v_sb.rearrange("p (s h c) -> p s h c", h=HPC, c=VROW)
            bvb3 = bvb_sb.rearrange("p (h c) -> p h c", h=HPC)
            cm3 = cmask_sb.rearrange("p (a q) -> p a q", a=2)

            def load_w(w_sb, w_d, n_w):
                sv = w_sb.rearrange("p (k n) -> p k n", n=n_w)
                dv = w_d.rearrange("(k p) n -> p k n", p=128)
                nc.sync.dma_start(sv, dv)

            # ---- projections ------------------------------------------
            def load_xch(x_d, n, dt, split=False):
                xch = wrk.tile([128, 8 * 512], dt, tag=f"xch_{dt}",
                               name="xch", bufs=2)
                sv = xch.rearrange("p (k s) -> p k s", k=8)
                dv = x_d.rearrange("(k p) s -> p k s", p=128)[
                    :, :, n * 512:(n + 1) * 512]
                if split:
                    for kt in range(0, 8, 2):
                        nc.sync.dma_start(sv[:, kt:kt + 2], dv[:, kt:kt + 2])
                else:
                    nc.sync.dma_start(sv, dv)
                return xch

            def proj_qk_unit(xch, w5, b_sb, dst3, n, m, i, on_act=False):
                # fp8 DoubleRow: out[p = W' col (slot,dm32), q 512]
                pp = ps.tile([128, 512], f32, tag="ps_small", name="pp",
                             bufs=2)
                x3 = xch.rearrange("p (k s) -> p k s", k=8)
                for kp in range(4):
                    nc.tensor.matmul(
                        pp[:],
                        w5[:, 2 * kp:2 * kp + 2, m, i, :],
                        x3[:, 2 * kp:2 * kp + 2, :],
                        start=(kp == 0),
                        stop=(kp == 3),
                        perf_mode=DR,
                    )
                dst = dst3[m][:, i, n * 512:(n + 1) * 512]
                if on_act:
                    nc.scalar.activation(
                        dst, pp[:], mybir.ActivationFunctionType.Identity,
                        bias=b_sb[:, 2 * m + i:2 * m + i + 1], scale=0.0625,
                    )
                else:
                    nc.vector.tensor_scalar(
                        dst, pp[:],
                        0.0625, b_sb[:, 2 * m + i:2 * m + i + 1], MULT, ADD,
                    )

            def proj_v_unit(xch, n, mi):
                st = n * 4 + mi
                pp = ps.tile([128, 512], f32, tag="ps_small", name="ppv",
                             bufs=2)
                for kt in range(8):
                    nc.tensor.matmul(
                        pp[:],
                        xch[:, kt * 512 + mi * 128:kt * 512 + (mi + 1) * 128],
                        wv_sb[:, kt * DM:(kt + 1) * DM],
                        start=(kt == 0),
                        stop=(kt == 7),
                    )
                nc.vector.tensor_add(
                    v4[:, st, :, 0:HD],
                    pp.rearrange("p (h c) -> p h c", h=HPC),
                    bvb3[:],
                )

            # -- A/B-split filler units: A emits PE matmuls into a fresh
            # -- psum tile, B (deferred one slot) moves psum out on DVE, so
            # -- B's semaphore wait never convoys the DVE queue.
            def qk_ab(x, w5, b_sb, dst3, n, m, i):
                cell = {}

                def fa():
                    pp = ps.tile([128, 512], f32, tag="ps_small", name="pp",
                                 bufs=2)
                    x3 = x.rearrange("p (k s) -> p k s", k=8)
                    for kp in range(4):
                        nc.tensor.matmul(
                            pp[:],
                            w5[:, 2 * kp:2 * kp + 2, m, i, :],
                            x3[:, 2 * kp:2 * kp + 2, :],
                            start=(kp == 0),
                            stop=(kp == 3),
                            perf_mode=DR,
                        )
                    cell["pp"] = pp

                def fb():
                    nc.vector.tensor_scalar(
                        dst3[m][:, i, n * 512:(n + 1) * 512], cell["pp"][:],
                        0.0625, b_sb[:, 2 * m + i:2 * m + i + 1], MULT, ADD,
                    )
                return (1024, fa), (0, fb)

            def v_ab(x, n, mi):
                st = n * 4 + mi
                cell = {}

                def fa():
                    pp = ps.tile([128, 512], f32, tag="ps_small", name="ppv",
                                 bufs=2)
                    for kt in range(8):
                        nc.tensor.matmul(
                            pp[:],
                            x[:, kt * 512 + mi * 128:kt * 512 + (mi + 1) * 128],
                            wv_sb[:, kt * DM:(kt + 1) * DM],
                            start=(kt == 0),
                            stop=(kt == 7),
                        )
                    cell["pp"] = pp

                def fb():
                    nc.vector.tensor_add(
                        v4[:, st, :, 0:HD],
                        cell["pp"].rearrange("p (h c) -> p h c", h=HPC),
                        bvb3[:],
                    )
                return (4096, fa), (0, fb)

            def outproj_ab(n, m8):
                cell = {}

                def fa():
                    pp = ps.tile([128, 512], f32, tag="ps_small", name="ppo",
                                 bufs=2)
                    for kt in range(4):
                        nc.tensor.matmul(
                            pp[:],
                            wo_sb[:, kt * D + m8 * 128:kt * D + (m8 + 1) * 128],
                            oT3[:, kt, n * 512:(n + 1) * 512],
                            start=(kt == 0),
                            stop=(kt == 3),
                        )
                    cell["pp"] = pp

                def fb():
                    ost = wrk.tile([128, 512], f32, tag="ost", name="ost",
                                   bufs=2)
                    nc.vector.tensor_copy(ost[:], cell["pp"][:])
                    nc.sync.dma_start(
                        outT_d[m8 * 128:(m8 + 1) * 128,
                               n * 512:(n + 1) * 512],
                        ost[:],
                    )
                return (2048, fa), (0, fb)

            def outproj_q_ab(m8, qt):
                cell = {}

                def fa():
                    pp = ps.tile([128, 128], f32, tag="ps_small", name="ppq",
                                 bufs=2)
                    for kt in range(4):
                        nc.tensor.matmul(
                            pp[:],
                            wo_sb[:, kt * D + m8 * 128:kt * D + (m8 + 1) * 128],
                            oT3[:, kt, qt * 128:(qt + 1) * 128],
                            start=(kt == 0),
                            stop=(kt == 3),
                        )
                    cell["pp"] = pp

                def fb():
                    ost = wrk.tile([128, 128], f32, tag="ostq", name="ostq",
                                   bufs=2)
                    nc.vector.tensor_copy(ost[:], cell["pp"][:])
                    nc.sync.dma_start(
                        outT_d[m8 * 128:(m8 + 1) * 128,
                               qt * 128:(qt + 1) * 128],
                        ost[:],
                    )
                return (853, fa), (0, fb)

            def interleave_ab(pairs):
                """[A1, A2, B1, A3, B2, ...]: each B lands one slot after
                its A so the psum tile's reader precedes the next writer."""
                out, prev_b = [], None
                for a, b in pairs:
                    out.append(("a", a))
                    if prev_b is not None:
                        out.append(("b", prev_b))
                    prev_b = b
                if prev_b is not None:
                    out.append(("b", prev_b))
                return out

            def proj_pairs(n):
                pairs = []
                xq = load_xch(xq_d, n, fp8)
                for m in range(2):
                    for i in range(2):
                        pairs.append(qk_ab(xq, wq5, bq_sb, qT3, n, m, i))
                xk = load_xch(xk_d, n, fp8)
                for m in range(2):
                    for i in range(2):
                        pairs.append(qk_ab(xk, wk5, bk_sb, kT3, n, m, i))
                xv = load_xch(xv_d, n, bf16)
                for mi in range(4):
                    pairs.append(v_ab(xv, n, mi))
                return pairs

            # ---- attention --------------------------------------------
            def attention_blocks(hp, qt, o_nat):
                """Generator yielding ACT-vs-PE slack (ns) after each block."""
                nblk = qt + 1
                heads = (2 * hp, 2 * hp + 1)
                m = hp // 2
                slots = (2 * (hp % 2), 2 * (hp % 2) + 1)
                aT = wrk.tile([128, 2 * S], bf16, tag="aT", name="aT", bufs=3)
                av = aT.rearrange("p (hh t q) -> p hh t q", hh=2, q=128)
                po = ps.tile([128, 1024], f32, tag="ps_o", name="po", bufs=1)
                pof = [po[:, 0:VROW], po[:, 512:512 + VROW]]

                nexp = (nblk + KT_PER_EXP - 1) // KT_PER_EXP
                cnts = [min(KT_PER_EXP, nblk - b * KT_PER_EXP)
                        for b in range(nexp)]

                def mm1_exp(blk, cnt):
                    psx = ps.tile([128, 1024], f32, tag="ps_x", name="psx",
                                  bufs=2)
                    px = psx.rearrange("p (hh j q) -> p hh j q", hh=2, j=4)
                    for j in range(cnt):
                        kt = blk * KT_PER_EXP + j
                        for hh in range(2):
                            sl = slots[hh]
                            nc.tensor.matmul(
                                px[:, hh, j, :],
                                kT3[m][32 * sl:32 * sl + 32, :,
                                       kt * 128:(kt + 1) * 128],
                                qT3[m][32 * sl:32 * sl + 32, :,
                                       qt * 128:(qt + 1) * 128],
                                start=True,
                                stop=True,
                                perf_mode=DR,
                                tile_position=(32 * sl, 0),
                            )
                    nc.scalar.activation(
                        av[:, :, blk * KT_PER_EXP:blk * KT_PER_EXP + cnt, :],
                        px[:, :, 0:cnt, :],
                        EXP,
                        scale=1.0 / 512.0,
                    )

                def mm2(blk, cnt):
                    for hh in range(2):
                        for j in range(cnt):
                            kt = blk * KT_PER_EXP + j
                            nc.tensor.matmul(
                                pof[hh],
                                av[:, hh, kt, :],
                                v4[:, kt, heads[hh], :],
                                start=(kt == 0),
                                stop=(kt == nblk - 1),
                                skip_group_check=True,
                            )

                tail_slack = 0.0
                for blk in range(nexp):
                    mm1_exp(blk, cnts[blk])
                    if blk == nexp - 1:
                        # mask the diagonal block (both heads, one op)
                        nc.vector.tensor_mul(av[:, :, qt, :], av[:, :, qt, :],
                                             cm3[:])
                    if blk > 0:
                        mm2(blk - 1, cnts[blk - 1])
                    pe_cy = (64 + 65) * 2 * cnts[blk]
                    slack = (cnts[blk] * 256 * _ACT_NS_PER_COL
                             + _ACT_OP_OVERHEAD_NS) - pe_cy * _PE_NS_PER_CY
                    if blk < nexp - 1:
                        yield slack
                    else:
                        tail_slack = slack
                mm2(nexp - 1, cnts[nexp - 1])

                rc = wrk.tile([128, 2], f32, tag="rc", name="rc", bufs=2)
                po_sums = po.rearrange("p (b c) -> p b c", c=512)[:, :, HD:HD + 1]
                nc.vector.reciprocal(rc[:], po_sums)
                for hh in range(2):
                    nc.vector.tensor_scalar_mul(
                        o_nat[:, heads[hh] * HD:(heads[hh] + 1) * HD],
                        pof[hh][:, 0:HD],
                        rc[:, hh:hh + 1],
                    )
                yield tail_slack

            def transpose_o(qt, o_nat):
                # 4 transposes into one psum bank, one strided DVE copy out
                pt = ps.tile([128, 512], bf16, tag="ps_small", bufs=2)
                for m in range(4):
                    nc.tensor.transpose(
                        pt[:, m * 128:(m + 1) * 128],
                        o_nat[:, m * 128:(m + 1) * 128],
                        ident_sb[:],
                    )
                nc.vector.tensor_copy(
                    oT3[:, :, qt * 128:(qt + 1) * 128],
                    pt.rearrange("p (m q) -> p m q", m=4),
                )

            # ---- output projection ------------------------------------
            def outproj_unit(n, m8):
                pp = ps.tile([128, 512], f32, tag="ps_small", name="ppo",
                             bufs=2)
                for kt in range(4):
                    nc.tensor.matmul(
                        pp[:],
                        wo_sb[:, kt * D + m8 * 128:kt * D + (m8 + 1) * 128],
                        oT3[:, kt, n * 512:(n + 1) * 512],
                        start=(kt == 0),
                        stop=(kt == 3),
                    )
                ost = wrk.tile([128, 512], f32, tag="ost", name="ost", bufs=2)
                nc.vector.tensor_copy(ost[:], pp[:])
                nc.sync.dma_start(
                    outT_d[m8 * 128:(m8 + 1) * 128, n * 512:(n + 1) * 512],
                    ost[:],
                )

            def outproj_unit_q(m8, qt):
                pp = ps.tile([128, 128], f32, tag="ps_small", name="ppq",
                             bufs=2)
                for kt in range(4):
                    nc.tensor.matmul(
                        pp[:],
                        wo_sb[:, kt * D + m8 * 128:kt * D + (m8 + 1) * 128],
                        oT3[:, kt, qt * 128:(qt + 1) * 128],
                        start=(kt == 0),
                        stop=(kt == 3),
                    )
                ost = wrk.tile([128, 128], f32, tag="ostq", name="ostq",
                               bufs=2)
                nc.vector.tensor_copy(ost[:], pp[:])
                nc.sync.dma_start(
                    outT_d[m8 * 128:(m8 + 1) * 128, qt * 128:(qt + 1) * 128],
                    ost[:],
                )

            # ---- schedule ---------------------------------------------
            # startup: weights + chunk-0 projections, DMA-ordered so the
            # first matmul's operands arrive first; psum->sbuf moves
            # alternate DVE/ACT while ACT has no exp work yet
            wq_v = wq_sb.rearrange("p (k n) -> p k n", n=DM)
            wq_dv = wq_d.rearrange("(k p) n -> p k n", p=128)
            nc.sync.dma_start(wq_v[:, 0:2], wq_dv[:, 0:2])
            nc.sync.dma_start(bq_sb[:], bq_d[:])
            xq0 = load_xch(xq_d, 0, fp8, split=True)
            nc.sync.dma_start(wq_v[:, 2:8], wq_dv[:, 2:8])
            for u, (m, i) in enumerate((m, i) for m in range(2)
                                       for i in range(2)):
                proj_qk_unit(xq0, wq5, bq_sb, qT3, 0, m, i, on_act=(u % 2))
            load_w(wk_sb, wk_d, DM)
            nc.sync.dma_start(bk_sb[:], bk_d[:])
            xk0 = load_xch(xk_d, 0, fp8)
            for u, (m, i) in enumerate((m, i) for m in range(2)
                                       for i in range(2)):
                proj_qk_unit(xk0, wk5, bk_sb, kT3, 0, m, i, on_act=(u % 2 == 0))
            load_w(wv_sb, wv_d, DM)
            nc.sync.dma_start(bvb_sb[:], bvb_d[:])
            nc.gpsimd.memset(v4[:, :, :, HD:HD + 1], 1.0)
            xv0 = load_xch(xv_d, 0, bf16)
            for mi in range(4):
                proj_v_unit(xv0, 0, mi)
            nc.sync.dma_start(cmask_sb[:], cmask_d[:])
            nc.sync.dma_start(ident_sb[:], ident_d[:])

            from collections import deque

            pending_t = [None]
            for n in range(4):
                pairs = []
                if n > 0:
                    pairs.extend(outproj_ab(n - 1, m8) for m8 in range(8))
                if n < 3:
                    pairs.extend(proj_pairs(n + 1))
                units = deque(interleave_ab(pairs))
                if n == 0:
                    load_w(wo_sb, wo_d, D)   # after chunk-1 x prefetches
                debt = [0.0]

                def pump(ns, units=units, debt=debt):
                    debt[0] = min(debt[0] + ns, 1100.0)
                    while units:
                        kind, (cy, fn) = units[0]
                        if kind == "a" and debt[0] <= 0:
                            break
                        units.popleft()
                        fn()
                        debt[0] -= cy * _PE_NS_PER_CY

                def drain_b(units=units):
                    while units and units[0][0] == "b":
                        units.popleft()[1][1]()

                for qt in range(4 * n, 4 * n + 4):
                    o_nat = wrk.tile([128, DM], bf16, tag="o_nat",
                                     name="o_nat", bufs=2)
                    nyield = 0
                    for hp in range(4):
                        for slack_ns in attention_blocks(hp, qt, o_nat):
                            nyield += 1
                            if nyield == 1 and pending_t[0] is not None:
                                # previous qt's transpose: deferred past this
                                # qt's first block so the o_nat wait is short,
                                # and ahead of any outproj filler that reads it
                                drain_b()
                                pending_t[0]()
                                pending_t[0] = None
                            pump(slack_ns)
                    def make_pending(qt=qt, o_nat=o_nat, n=n, units=units):
                        def fn():
                            transpose_o(qt, o_nat)
                            if n == 3:
                                units.extend(
                                    interleave_ab([outproj_q_ab(m8, qt)
                                                   for m8 in range(8)]))
                        return fn

                    pending_t[0] = make_pending()
                    if n == 3 and qt == 15:
                        drain_b()
                        pending_t[0]()
                        pending_t[0] = None
                        while units:
                            units.popleft()[1][1]()
                while units:
                    units.popleft()[1][1]()

    nc.compile()
    return nc


def _get_program():
    if "nc" not in _CACHE:
        _CACHE["nc"] = _build_program()
    return _CACHE["nc"]


def _fold_perm():
    """Column permutation for the folded Q/K layout.

    W' column (m, i, c) <- original in-group dim (4m + c//32)*64 + i*32 + c%32.
    """
    perm = np.empty(DM, dtype=np.int64)
    idx = 0
    for m in range(2):
        for i in range(2):
            for c in range(128):
                perm[idx] = (4 * m + c // 32) * HD + i * 32 + c % 32
                idx += 1
    return perm


def _make_in_maps(query, key, value, Wq, bq, Wk, bk, Wv, bv, Wo):
    import ml_dtypes

    bf16 = ml_dtypes.bfloat16
    e4m3 = ml_dtypes.float8_e4m3
    cmask = np.tile(np.triu(np.ones((128, 128), dtype=np.float32)),
                    (1, 2)).astype(bf16)
    ident = np.eye(128, dtype=np.float32).astype(bf16)
    perm = _fold_perm()
    in_maps = []
    xq8 = [np.ascontiguousarray(query[b].T * 16.0).astype(e4m3)
           for b in range(B)]
    xk8 = [np.ascontiguousarray(key[b].T * 16.0).astype(e4m3)
           for b in range(B)]
    xvb = [np.ascontiguousarray(value[b].T).astype(bf16) for b in range(B)]
    for c in range(NCORES):
        b, g = c // 2, c % 2
        sl = slice(g * DM, (g + 1) * DM)
        wq_g, bq_g = Wq[:, sl][:, perm], bq[sl][perm]
        wk_g, bk_g = Wk[:, sl][:, perm], bk[sl][perm]
        in_maps.append({
            "xq": xq8[b],
            "xk": xk8[b],
            "xv": xvb[b],
            "wq": np.ascontiguousarray(wq_g * 8.0).astype(e4m3),
            "wk": np.ascontiguousarray(wk_g * 8.0).astype(e4m3),
            "wv": np.ascontiguousarray(Wv[:, sl]).astype(bf16),
            "bq": np.ascontiguousarray((bq_g * 8.0).reshape(4, 128).T
                                       .astype(np.float32)),
            "bk": np.ascontiguousarray((bk_g * 8.0).reshape(4, 128).T
                                       .astype(np.float32)),
            "bvb": np.ascontiguousarray(
                np.broadcast_to(bv[sl], (128, DM)).astype(np.float32)
            ),
            "wo": np.ascontiguousarray(Wo[sl, :]).astype(bf16),
            "cmask": cmask,
            "ident": ident,
        })
    return in_maps


def _run_spmd(in_maps, trace=False):
    from concourse import bass_utils

    nc = _get_program()
    return bass_utils.run_bass_kernel_spmd(
        nc, in_maps, core_ids=list(range(NCORES)), trace=trace
    )


def _assemble(res, bo):
    out = np.empty((B, S, D), dtype=np.float32)
    bo32 = np.asarray(bo, dtype=np.float32)
    for b in range(B):
        out[b] = (
            res.results[2 * b]["outT"].T
            + res.results[2 * b + 1]["outT"].T
            + bo32
        )
    return out


def _numpy_fallback(query, key, value, mask, Wq, bq, Wk, bk, Wv, bv, Wo, bo):
    """Correct (slow) host path for non-causal masks; never used when the
    mask is the reference's tril."""
    def split_heads(x):
        b, s, _ = x.shape
        return x.reshape(b, s, H, HD).transpose(0, 2, 1, 3)

    q = split_heads(query @ Wq + bq)
    k = split_heads(key @ Wk + bk)
    v = split_heads(value @ Wv + bv)
    nb = query.shape[0]
    out = np.empty((nb, H, S, HD), dtype=np.float32)
    for b in range(nb):
        mb = np.asarray(mask[b, 0]) != 0
        for h in range(H):
            s = (q[b, h] @ k[b, h].T) / math.sqrt(HD)
            s = np.where(mb, s, -np.inf)
            s -= s.max(axis=-1, keepdims=True)
            e = np.exp(s)
            a = e / e.sum(axis=-1, keepdims=True)
            a *= mb
            out[b, h] = a @ v[b, h]
    out = out.transpose(0, 2, 1, 3).reshape(nb, -1, D)
    return (out @ Wo + bo).astype(np.float32)


def kernel(query, key, value, mask, Wq, bq, Wk, bk, Wv, bv, Wo, bo):
    query = np.asarray(query, dtype=np.float32)
    key = np.asarray(key, dtype=np.float32)
    value = np.asarray(value, dtype=np.float32)
    mask = np.asarray(mask)
    Wq = np.asarray(Wq, dtype=np.float32)
    bq = np.asarray(bq, dtype=np.float32)
    Wk = np.asarray(Wk, dtype=np.float32)
    bk = np.asarray(bk, dtype=np.float32)
    Wv = np.asarray(Wv, dtype=np.float32)
    bv = np.asarray(bv, dtype=np.float32)
    Wo = np.asarray(Wo, dtype=np.float32)
    bo = np.asarray(bo, dtype=np.float32)

    causal = np.array_equal(
        np.asarray(mask[0, 0], dtype=np.int32),
        np.tril(np.ones((S, S), dtype=np.int32)),
    ) and all(np.array_equal(mask[b], mask[0]) for b in range(1, mask.shape[0]))
    if not causal:
        return _numpy_fallback(
            query, key, value, mask, Wq, bq, Wk, bk, Wv, bv, Wo, bo
        )

    in_maps = _make_in_maps(query, key, value, Wq, bq, Wk, bk, Wv, bv, Wo)
    res = _run_spmd(in_maps)
    return _assemble(res, bo)
